# revision 5
# baseline (speedup 1.0000x reference)
"""3-layer GCN (message passing) + sum-pool + MLP head on 8 Trainium2 cores.

v6: template-trimmed variable chunk counts + 4-queue SWDGE gathers (parallel descriptor generation across Q7 CPU
pairs), plain one-hots shared by all 3 layers (L3 folds d_inv[dst] as a
per-partition column scale on a transposed aggregation instead of per-edge
one-hot values), L3 sum-pool via SBUF accumulator + ones-matmul.

Strategy (all shapes hardcoded; self-contained):
  - Host graph preprocessing: permute nodes into 392 blocks of 128 (49
    blocks/core); nodes split into two gather halves (int16 idx limit) with
    per-block-half edge capacity balanced by a greedy packer.
  - Layer 1 aggregates x*d_inv directly (linearity: A(xW0) = (Ax)W0), so the
    L1 gather table is a replicated input - no dense pre-pass, no AllGather.
  - Tables are bf16 [*, 128]; one-hot routing matrices are exact 0/1 bf16;
    GCN normalization is folded into d_inv^2 epilogues (L1/L2) and a
    d_inv[dst] per-partition scale on L3's transposed aggregation.
  - Node-sharded dense outputs are AllGather'd between layers; pooled vector
    is AllReduce'd; the tiny MLP head runs replicated on-device.
"""
import sys

import numpy as np

for _p in ("/opt/trn_rl_repo", "/root/.axon_site/_ro/trn_rl_repo"):
    if _p not in sys.path:
        sys.path.append(_p)

import ml_dtypes

import concourse.bacc as bacc
import concourse.bass as bass
import concourse.mybir as mybir
import concourse.tile as tile
from concourse.bass_utils import run_bass_kernel_spmd

# ---------------------------------------------------------------- constants
N = 50000                 # real nodes
P = 128
NB = 392                  # blocks (of 128 node slots)
NP = NB * P               # padded nodes = 50176
NCORES = 8
BPC = NB // NCORES        # 49 blocks per core
ROWS_PC = BPC * P         # 6272 rows per core shard
NHB = NB // 2             # 196 blocks per half
H = NHB * P               # 25088 = gather-half split (int16 idx limit)
CAP = 1152                # edge capacity per block per source-half
NCH = CAP // P            # 9 chunks per half
CHB = 2 * NCH             # 18 edge chunks per block
BATCH_SIZES = [7, 7, 7, 7, 7, 7, 5, 2]  # gather batching; small tail batch
SPLIT = 4                            # sub-gather split: 4 blocks + 3 blocks
IDX_COLS = 2 * (CAP // 16) * BPC     # 7056 idx columns (int16, wrapped by 16)
FW = 128                  # stored table width (bf16)
NQ = 4                    # SWDGE queues (round-robin gathers)

_CACHED = {}
BF16 = ml_dtypes.bfloat16


# ------------------------------------------------------------- host prepro
def _balance_blocks(a_w, b_w, nblocks, cap):
    """Greedy-pack nodes (with per-node loads a_w/b_w) into blocks of <=128
    nodes with per-half loads <= cap. Returns block id per node position."""
    order = np.argsort(-(a_w + b_w), kind="stable")
    la = np.zeros(nblocks, np.int64)
    lb = np.zeros(nblocks, np.int64)
    cnt = np.zeros(nblocks, np.int64)
    out = np.empty(len(a_w), np.int64)
    for i in order:
        na = la + a_w[i]
        nb_ = lb + b_w[i]
        score = np.maximum(na, nb_).astype(np.float64)
        score[(cnt >= P) | (na > cap) | (nb_ > cap)] = np.inf
        j = int(np.argmin(score))
        assert np.isfinite(score[j]), "block packing infeasible; raise CAP"
        out[i] = j
        la[j] = na[j]
        lb[j] = nb_[j]
        cnt[j] += 1
    return out


def _preprocess(x, edge_index):
    src = np.asarray(edge_index[0], np.int64)
    dst = np.asarray(edge_index[1], np.int64)

    deg = np.bincount(dst, minlength=N).astype(np.float64)
    d_inv = 1.0 / np.sqrt(deg + 1.0)

    # ---- split nodes into halves balancing out-edge (as-source) mass
    out_w = np.bincount(src, minlength=N)
    order = np.argsort(-out_w, kind="stable")
    half = np.zeros(N, np.int8)
    tot = [0, 0]
    cnti = [0, 0]
    for i in order:
        h_ = 0 if (tot[0] <= tot[1] and cnti[0] < H) or cnti[1] >= H else 1
        half[i] = h_
        tot[h_] += out_w[i]
        cnti[h_] += 1

    # ---- per-node in-loads split by source half
    sh = half[src]
    a_in = np.bincount(dst[sh == 0], minlength=N)
    b_in = np.bincount(dst[sh == 1], minlength=N)

    # ---- pack each half's nodes into its 196 blocks
    _bal_cache = {}
    for h_ in (0, 1):
        nodes = np.nonzero(half == h_)[0]
        blk = _balance_blocks(a_in[nodes], b_in[nodes], NHB, CAP)
        _bal_cache[h_] = (blk,)

    # ---- block loads per (half, block, srchalf); redeal blocks by load
    pre_blk = np.empty(N, np.int64)
    pre_pos = np.empty(N, np.int64)
    loads = np.zeros((2, NHB, 2), np.int64)
    for h_ in (0, 1):
        nodes = np.nonzero(half == h_)[0]
        blk = _bal_cache[h_][0]
        o2 = np.argsort(blk, kind="stable")
        sb = blk[o2]
        grp_start = np.searchsorted(sb, np.arange(NHB), side="left")
        pos_in_grp = np.arange(len(nodes)) - grp_start[sb]
        pre_blk[nodes[o2]] = sb
        pre_pos[nodes[o2]] = pos_in_grp
        np.add.at(loads[h_], (blk, 0), a_in[nodes])
        np.add.at(loads[h_], (blk, 1), b_in[nodes])

    # snake-deal each half's blocks across its 4 cores by total load
    CPH = NCORES // 2                  # cores per half
    SPH = NHB // CPH                   # 49 block slots per core
    new_g = np.empty((2, NHB), np.int64)
    slot_loads = np.zeros((NCORES, BPC, 2), np.int64)
    for h_ in (0, 1):
        rank = np.empty(NHB, np.int64)
        rank[np.argsort(-loads[h_].sum(1), kind="stable")] = np.arange(NHB)
        srow = rank // CPH
        scol = np.where(srow % 2 == 0, rank % CPH, CPH - 1 - rank % CPH)
        new_g[h_] = scol * SPH + srow
        for ob in range(NHB):
            c = h_ * CPH + new_g[h_][ob] // SPH
            j = new_g[h_][ob] % SPH
            slot_loads[c, j, 0] = loads[h_][ob, 0]
            slot_loads[c, j, 1] = loads[h_][ob, 1]

    # template: chunks per (core-local slot j, src half h) = max over cores
    n_ch = np.maximum(
        1, np.ceil(slot_loads.max(axis=0) / P).astype(np.int64))  # [BPC, 2]
    assert n_ch.sum(1).max() <= CHB
    tmpl = tuple(map(tuple, n_ch))

    # final node positions
    perm_pos = np.empty(N, np.int64)
    for h_ in (0, 1):
        nodes = np.nonzero(half == h_)[0]
        perm_pos[nodes] = (h_ * NHB + new_g[h_][pre_blk[nodes]]) * P \
            + pre_pos[nodes]

    # ---- remap edges, group by (dst block, src half)
    psrc = perm_pos[src]
    pdst = perm_pos[dst]
    eb = pdst >> 7              # dst block
    es = pdst & 127             # dst slot
    eh = (psrc >= H).astype(np.int64)
    eidx = psrc - eh * H        # gather idx within half

    key = eb * 2 + eh
    order_e = np.argsort(key, kind="stable")
    key_s = key[order_e]
    cnts = np.bincount(key_s, minlength=NB * 2)
    starts = np.concatenate([[0], np.cumsum(cnts)[:-1]])
    pos = np.arange(len(key_s)) - starts[key_s]

    # ---- variable-chunk layout tables (core-uniform template)
    nchA = n_ch[:, 0]
    nchB = n_ch[:, 1]
    tch = nchA + nchB
    col0 = np.concatenate([[0], np.cumsum(tch)[:-1]])      # dstloc col base
    TCHT = int(tch.sum())
    bs_arr = np.array(BATCH_SIZES)
    blk2batch = np.repeat(np.arange(len(bs_arr)), bs_arr)
    batch_blk0 = np.concatenate([[0], np.cumsum(bs_arr)[:-1]])
    # idx col layout: per batch: A-half cols then B-half cols
    colsA = np.array([nchA[batch_blk0[t]:batch_blk0[t] + bs_arr[t]].sum() * 8
                      for t in range(len(bs_arr))])
    colsB = np.array([nchB[batch_blk0[t]:batch_blk0[t] + bs_arr[t]].sum() * 8
                      for t in range(len(bs_arr))])
    batch_col0 = np.concatenate([[0], np.cumsum(colsA + colsB)[:-1]])
    IDXCT = int((colsA + colsB).sum())
    # chunk row base of block within its (batch, half) gather tile
    qbase = np.zeros((BPC, 2), np.int64)
    for t in range(len(bs_arr)):
        accA = accB = 0
        for k in range(bs_arr[t]):
            j = batch_blk0[t] + k
            qbase[j, 0] = accA
            qbase[j, 1] = accB
            accA += nchA[j] * P
            accB += nchB[j] * P

    assert cnts.max() <= CAP
    # per-cell capacity check vs template
    g_eb = eb[order_e]
    g_eh = eh[order_e]
    g_core = g_eb // BPC
    g_j = g_eb % BPC
    for c_ in range(NCORES):
        pass  # template derived from max loads; cnts <= n_ch*P by constr.

    # ---- fill per-core device arrays
    idxs = np.zeros((NCORES, 16, IDXCT), np.int16)
    dstloc = np.full((NCORES, P, TCHT), 999.0, np.float32)

    g_batch = blk2batch[g_j]

    col_dw = col0[g_j] + g_eh * nchA[g_j] + pos // P
    dstloc[g_core, pos % P, col_dw] = es[order_e]

    call_off = batch_col0[g_batch] + g_eh * colsA[g_batch]
    q = qbase[g_j, g_eh] + pos
    idxs[g_core, q % 16, call_off + q // 16] = eidx[order_e]
    idxs_full = np.tile(idxs, (1, 8, 1))  # replicate to 128 partitions

    # ---- bf16 L1 gather table: xg[perm(n), 0:14] = x[n] * d_inv[n]
    xg = np.zeros((NP, FW), BF16)
    xg[perm_pos, :14] = (np.asarray(x, np.float64)
                         * d_inv[:, None]).astype(BF16)
    xg_own = xg.reshape(NCORES, ROWS_PC, FW)

    # ---- per-slot d_inv arrays [core][slot, block]
    dinv1 = np.zeros((NCORES, P, BPC), np.float32)
    dinv2 = np.zeros((NCORES, P, BPC), np.float32)
    blk_all = perm_pos >> 7
    slot_all = perm_pos & 127
    dinv1[blk_all // BPC, slot_all, blk_all % BPC] = d_inv
    dinv2[blk_all // BPC, slot_all, blk_all % BPC] = d_inv * d_inv
    return (xg, xg_own, idxs_full, dstloc, dinv1, dinv2, tmpl)


# ------------------------------------------------------------ device build
def _layout(tmpl):
    n_ch = np.array(tmpl, np.int64)            # [BPC, 2]
    nchA, nchB = n_ch[:, 0], n_ch[:, 1]
    tch = nchA + nchB
    col0 = np.concatenate([[0], np.cumsum(tch)[:-1]])
    bs_arr = np.array(BATCH_SIZES)
    batch_blk0 = np.concatenate([[0], np.cumsum(bs_arr)[:-1]])
    colsA = np.array([nchA[batch_blk0[t]:batch_blk0[t] + bs_arr[t]].sum() * 8
                      for t in range(len(bs_arr))])
    colsB = np.array([nchB[batch_blk0[t]:batch_blk0[t] + bs_arr[t]].sum() * 8
                      for t in range(len(bs_arr))])
    batch_col0 = np.concatenate([[0], np.cumsum(colsA + colsB)[:-1]])
    qbase = np.zeros((BPC, 2), np.int64)
    for t in range(len(bs_arr)):
        accA = accB = 0
        for k in range(bs_arr[t]):
            j = batch_blk0[t] + k
            qbase[j, 0] = accA
            qbase[j, 1] = accB
            accA += nchA[j] * P
            accB += nchB[j] * P
    return (nchA, nchB, tch, col0, bs_arr, batch_blk0, colsA, colsB,
            batch_col0, qbase, int(tch.sum()), int((colsA + colsB).sum()))


def _build_kernel(tmpl):
    (nchA, nchB, tch, col0, bs_arr, batch_blk0, colsA, colsB,
     batch_col0, qbase, TCHT, IDXCT) = _layout(tmpl)
    nc = bacc.Bacc("TRN2", target_bir_lowering=False, debug=False,
                   num_swdge_queues=NQ)
    dt = mybir.dt

    xg = nc.dram_tensor("xg", [NP, FW], dt.bfloat16, kind="ExternalInput")
    xgo = nc.dram_tensor("xgo", [ROWS_PC, FW], dt.bfloat16, kind="ExternalInput")
    w0 = nc.dram_tensor("w0", [14, 128], dt.float32, kind="ExternalInput")
    w1 = nc.dram_tensor("w1", [128, 128], dt.float32, kind="ExternalInput")
    w2p = nc.dram_tensor("w2p", [128, 32], dt.float32, kind="ExternalInput")
    fc11w = nc.dram_tensor("fc11w", [32, 16], dt.float32, kind="ExternalInput")
    fc11b = nc.dram_tensor("fc11b", [16, 1], dt.float32, kind="ExternalInput")
    fc12w = nc.dram_tensor("fc12w", [16, 1], dt.float32, kind="ExternalInput")
    fc12b = nc.dram_tensor("fc12b", [1, 1], dt.float32, kind="ExternalInput")
    iotab = nc.dram_tensor("iotab", [P, CHB * P], dt.float32, kind="ExternalInput")
    ident = nc.dram_tensor("ident", [P, P], dt.bfloat16, kind="ExternalInput")
    ones = nc.dram_tensor("ones", [P, 1], dt.float32, kind="ExternalInput")
    dinv1 = nc.dram_tensor("dinv1", [P, BPC], dt.float32, kind="ExternalInput")
    dinv2 = nc.dram_tensor("dinv2", [P, BPC], dt.float32, kind="ExternalInput")
    idxs = nc.dram_tensor("idxs", [P, IDXCT], dt.int16, kind="ExternalInput")
    dstloc = nc.dram_tensor("dstloc", [P, TCHT], dt.float32, kind="ExternalInput")
    out = nc.dram_tensor("out", [1, 1], dt.float32, kind="ExternalOutput")

    maxwA = int(max(colsA)) * 16 // P     # max ga tile chunk count per batch
    maxwB = int(max(colsB)) * 16 // P

    qctr = [0]

    def next_q():
        q = qctr[0] % NQ
        qctr[0] += 1
        return q

    with tile.TileContext(nc) as tc:
        with (
            tc.tile_pool(name="const", bufs=1) as cst,
            tc.tile_pool(name="ga", bufs=4) as gap,
            tc.tile_pool(name="gb", bufs=3) as gbp,
            tc.tile_pool(name="gs", bufs=2) as gsp,
            tc.tile_pool(name="oh", bufs=8) as ohp,
            tc.tile_pool(name="rl", bufs=3) as rlp,
            tc.tile_pool(name="st", bufs=2) as stp,
            tc.tile_pool(name="misc", bufs=1) as msc,
            tc.tile_pool(name="psA", bufs=2, space="PSUM") as psa,
            tc.tile_pool(name="psX", bufs=2, space="PSUM") as psx,
            tc.tile_pool(name="psD", bufs=2, space="PSUM") as psd,
            tc.tile_pool(name="psP", bufs=1, space="PSUM") as psp,
            tc.tile_pool(name="dram", bufs=1, space="DRAM") as drm,
        ):
            # resident constants
            idxs_t = cst.tile([P, IDXCT], dt.int16)
            dstloc_t = cst.tile([P, TCHT], dt.float32)
            iotab_t = cst.tile([P, CHB * P], dt.float32)
            ident_t = cst.tile([P, P], dt.bfloat16)
            ones_t = cst.tile([P, 1], dt.float32)
            dinv1_t = cst.tile([P, BPC], dt.float32)
            dinv2_t = cst.tile([P, BPC], dt.float32)
            w0_t = cst.tile([14, 128], dt.float32)
            w1_t = cst.tile([128, 128], dt.float32)
            w2p_t = cst.tile([128, 32], dt.float32)
            fc11w_t = cst.tile([32, 16], dt.float32)
            fc11b_t = cst.tile([16, 1], dt.float32)
            fc12w_t = cst.tile([16, 1], dt.float32)
            fc12b_t = cst.tile([1, 1], dt.float32)
            for t_, d_ in (
                (idxs_t, idxs), (dstloc_t, dstloc),
                (iotab_t, iotab), (ident_t, ident), (ones_t, ones),
                (dinv1_t, dinv1), (dinv2_t, dinv2),
                (w0_t, w0), (w1_t, w1), (w2p_t, w2p),
                (fc11w_t, fc11w), (fc11b_t, fc11b), (fc12w_t, fc12w),
                (fc12b_t, fc12b),
            ):
                nc.sync.dma_start(t_[:], d_[:])

            # internal DRAM (bf16 tables)
            g2s_t = drm.tile([ROWS_PC, FW], dt.bfloat16)
            g2_t = drm.tile([NP, FW], dt.bfloat16, addr_space="Shared")
            g3s_t = drm.tile([ROWS_PC, FW], dt.bfloat16)
            g3_t = drm.tile([NP, FW], dt.bfloat16, addr_space="Shared")
            pool_in = drm.tile([32, 1], dt.float32)
            pool_out = drm.tile([32, 1], dt.float32)

            acc3 = msc.tile([P, 32], dt.float32)
            nc.vector.memset(acc3[:], 0.0)

            def gather_batch(h_src, t, tag_a, tag_b):
                bs = int(bs_arr[t])
                b0 = int(batch_blk0[t])
                ic0 = int(batch_col0[t])
                nA = int(colsA[t]) * 16 // P      # ga chunk count
                nB = int(colsB[t]) * 16 // P
                ga = gap.tile([P, maxwA, FW], dt.bfloat16, tag=tag_a)
                gb = gbp.tile([P, maxwB, FW], dt.bfloat16, tag=tag_b)
                s0 = SPLIT if bs > SPLIT else bs
                for (tile_, base, ncount, hc0) in (
                    (ga, 0, nA, ic0),
                    (gb, 1, nB, ic0 + int(colsA[t])),
                ):
                    lo, hi = (0, H) if base == 0 else (H, NP)
                    carr = nchA if base == 0 else nchB
                    n0 = int(carr[b0:b0 + s0].sum())      # chunks in sub 0
                    r0 = n0 * P
                    nall = ncount * P
                    nc.gpsimd.dma_gather(
                        tile_[:, 0:n0, :], h_src[lo:hi, :],
                        idxs_t[:, hc0:hc0 + r0 // 16],
                        r0, r0, FW, single_packet=False, queue_num=next_q())
                    if nall > r0:
                        nc.gpsimd.dma_gather(
                            tile_[:, n0:ncount, :], h_src[lo:hi, :],
                            idxs_t[:, hc0 + r0 // 16:hc0 + nall // 16],
                            nall - r0, nall - r0, FW,
                            single_packet=False, queue_num=next_q())
                return ga, gb

            def layer(lnum, h_src, h_self, h_shard):
                for t in range(len(BATCH_SIZES)):
                    bs = int(bs_arr[t])
                    b0 = int(batch_blk0[t])
                    ga, gb = gather_batch(h_src, t, "ga", "gb")
                    gs = gsp.tile([P, bs, FW], dt.bfloat16, tag="gs")
                    nc.sync.dma_start(
                        gs[:],
                        h_self[b0 * P : (b0 + bs) * P, :].rearrange(
                            "(g p) f -> p g f", p=P))
                    dw = FW if lnum == 1 else 32
                    dstage = None
                    if lnum != 3:
                        dstage = stp.tile([P, bs, dw], dt.bfloat16, tag="dnst")
                    for k in range(bs):
                        j = b0 + k
                        na, nb_, tc_ = int(nchA[j]), int(nchB[j]), int(tch[j])
                        qa = int(qbase[j, 0]) // P
                        qb = int(qbase[j, 1]) // P
                        agg = psa.tile([P, P], dt.float32, tag="agg")
                        iview = iotab_t[:].rearrange(
                            "p (c j) -> p c j", j=P)[:, 0:tc_, :]
                        dview = dstloc_t[:, col0[j] : col0[j] + tc_].rearrange(
                            "p (c o) -> p c o", o=1).to_broadcast([P, tc_, P])
                        ohb = ohp.tile([P, CHB, P], dt.bfloat16, tag="ohb")
                        nc.vector.tensor_tensor(
                            ohb[:, 0:tc_, :], iview, dview,
                            mybir.AluOpType.is_equal)
                        if lnum == 3:
                            for c in range(tc_):
                                g, cc = (ga, qa + c) if c < na else                                     (gb, qb + c - na)
                                nc.tensor.matmul(
                                    agg[:, 0:32], ohb[:, c, :],
                                    g[:, cc, 0:32],
                                    start=(c == 0), stop=False)
                            nc.tensor.matmul(
                                agg[:, 0:32], ident_t[:], gs[:, k, 0:32],
                                start=False, stop=True)
                            r3t = rlp.tile([P, 32], dt.float32, tag="r3t")
                            nc.vector.tensor_scalar(
                                r3t[:], agg[:, 0:32],
                                dinv1_t[:, j : j + 1], 0.0,
                                mybir.AluOpType.mult, mybir.AluOpType.max)
                            nc.vector.tensor_tensor(
                                acc3[:], acc3[:], r3t[:],
                                mybir.AluOpType.add)
                            continue
                        for c in range(tc_):
                            g, cc = (ga, qa + c) if c < na else                                 (gb, qb + c - na)
                            nc.tensor.matmul(
                                agg[:], g[:, cc, :], ohb[:, c, :],
                                start=(c == 0), stop=False)
                        nc.tensor.matmul(
                            agg[:], gs[:, k, :], ident_t[:],
                            start=False, stop=True)

                        if lnum == 1:
                            axs = rlp.tile([14, P], dt.float32, tag="axs")
                            nc.vector.tensor_copy(axs[:], agg[0:14, :])
                            zt = psx.tile([P, P], dt.float32, tag="zt")
                            nc.tensor.matmul(
                                zt[:], w0_t[:], axs[:], start=True, stop=True)
                            rT = rlp.tile([P, P], dt.float32, tag="rT")
                            nc.scalar.activation(
                                rT[:], zt[:],
                                mybir.ActivationFunctionType.Relu)
                            h_ps = psd.tile([P, FW], dt.float32, tag="dnps")
                            nc.tensor.matmul(
                                h_ps[:], rT[:], w1_t[:], start=True, stop=True)
                            nc.vector.tensor_scalar(
                                dstage[:, k, :], h_ps[:],
                                dinv2_t[:, j : j + 1], None,
                                mybir.AluOpType.mult)
                        else:
                            rT = rlp.tile([P, P], dt.float32, tag="rT")
                            nc.scalar.activation(
                                rT[:], agg[:],
                                mybir.ActivationFunctionType.Relu)
                            h_ps = psd.tile([P, 32], dt.float32, tag="dnps")
                            nc.tensor.matmul(
                                h_ps[:], rT[:], w2p_t[:], start=True, stop=True)
                            nc.vector.tensor_scalar(
                                dstage[:, k, :], h_ps[:],
                                dinv2_t[:, j : j + 1], None,
                                mybir.AluOpType.mult)
                    if lnum != 3:
                        r0 = b0 * P
                        nc.sync.dma_start(
                            h_shard[r0 : r0 + bs * P, 0:dw].rearrange(
                                "(g p) f -> p g f", p=P),
                            dstage[:, :bs, :])

            # L1 (x-aggregation)
            layer(1, xg, xgo, g2s_t)
            nc.gpsimd.collective_compute(
                "AllGather", mybir.AluOpType.bypass,
                replica_groups=[list(range(NCORES))],
                ins=[g2s_t.opt()], outs=[g2_t.opt()])
            # L2
            layer(2, g2_t, g2s_t, g3s_t)
            nc.gpsimd.collective_compute(
                "AllGather", mybir.AluOpType.bypass,
                replica_groups=[list(range(NCORES))],
                ins=[g3s_t.opt()], outs=[g3_t.opt()])
            # L3 + pooling (acc3 accumulated per-slot across blocks)
            layer(3, g3_t, g3s_t, None)
            pooled_ps = psp.tile([32, 1], dt.float32, tag="pool")
            nc.tensor.matmul(
                pooled_ps[:], acc3[:], ones_t[:], start=True, stop=True)
            pooled = msc.tile([32, 1], dt.float32)
            nc.vector.tensor_copy(pooled[:], pooled_ps[:])

            nc.sync.dma_start(pool_in[:], pooled[:])
            nc.gpsimd.collective_compute(
                "AllReduce", mybir.AluOpType.add,
                replica_groups=[list(range(NCORES))],
                ins=[pool_in.opt()], outs=[pool_out.opt()])
            pooled_g = msc.tile([32, 1], dt.float32)
            nc.sync.dma_start(pooled_g[:], pool_out[:])
            ps16 = psp.tile([16, 1], dt.float32, tag="mlp")
            nc.tensor.matmul(ps16[:], fc11w_t[:], pooled_g[:], start=True, stop=True)
            a16 = msc.tile([16, 1], dt.float32)
            nc.scalar.activation(
                a16[:], ps16[:], mybir.ActivationFunctionType.Relu,
                bias=fc11b_t[:])
            ps1 = psp.tile([1, 1], dt.float32, tag="mlp")
            nc.tensor.matmul(ps1[:], fc12w_t[:], a16[:], start=True, stop=True)
            o1 = msc.tile([1, 1], dt.float32)
            nc.scalar.activation(
                o1[:], ps1[:], mybir.ActivationFunctionType.Identity,
                bias=fc12b_t[:])
            nc.sync.dma_start(out[:], o1[:])

    nc.compile()
    return nc


def _get_nc(tmpl):
    if tmpl not in _CACHED:
        _CACHED[tmpl] = _build_kernel(tmpl)
    return _CACHED[tmpl]


def _make_in_maps(inputs):
    x = np.asarray(inputs["x"], np.float32)
    edge_index = np.asarray(inputs["edge_index"])
    xg, xg_own, idxs, dstloc, dinv1, dinv2, tmpl = _preprocess(x, edge_index)

    w2p = np.asarray(inputs["W2"], np.float32)
    common = {
        "xg": xg,
        "w0": np.asarray(inputs["W0"], np.float32),
        "w1": np.asarray(inputs["W1"], np.float32),
        "w2p": w2p,
        "fc11w": np.asarray(inputs["fc11_w"], np.float32),
        "fc11b": np.asarray(inputs["fc11_b"], np.float32).reshape(16, 1),
        "fc12w": np.asarray(inputs["fc12_w"], np.float32),
        "fc12b": np.asarray(inputs["fc12_b"], np.float32).reshape(1, 1),
        "iotab": np.tile(np.arange(P, dtype=np.float32), (P, CHB)),
        "ident": np.eye(P, dtype=BF16),
        "ones": np.ones((P, 1), np.float32),
    }
    return [
        {**common, "xgo": np.ascontiguousarray(xg_own[c]), "idxs": idxs[c],
         "dstloc": dstloc[c],
         "dinv1": dinv1[c], "dinv2": dinv2[c]}
        for c in range(NCORES)
    ], tmpl


def run(trace=False, _inputs=None, **inputs):
    if _inputs is not None:
        inputs = _inputs
    in_maps, tmpl = _make_in_maps(inputs)
    nc = _get_nc(tmpl)
    res = run_bass_kernel_spmd(
        nc, in_maps, core_ids=list(range(NCORES)), trace=trace)
    y = np.asarray(res.results[0]["out"], np.float32).reshape(1)
    return y, res


def kernel(**inputs) -> np.ndarray:
    y, _ = run(**inputs)
    return y
